# revision 6
# baseline (speedup 1.0000x reference)
"""C2Q attention kernel for 8 TRN2 NeuronCores.

Math (per batch):
    u      = (o_q @ W.T + b) / sqrt(H)          [Tq, H]
    score  = o_c @ u.T                           [Tc, Tq]
    prob   = softmax_j(score masked at j>=q_len) [Tc, Tq]
    out    = (prob * (i < c_len)) @ o_q          [Tc, H]

Device layout choices (everything lands K-on-partitions with zero on-chip
transposes of activations):
    u computed as [o, j]  (lhsT = W.T[h, o] tile, rhs = o_qT[h, j])
    score computed TRANSPOSED e=[j, i] (lhsT = u[o, j-block], rhs = o_cT[o, i])
    exp via ACT with per-partition bias qb[j] in {0, -1e7}: masked -> exactly 0
    denominator d[1, i] = ones[j,1].T @ e  (matmul partition-reduce)
    1/d transposed to columns via K=1 matmuls, folded into context eviction
    context [i, h] = e[j, i-block].T @ o_q[j, h]   (natural output layout)
c_len row masking is applied host-side (those rows are zeroed, never read).
"""

import os
import sys

import numpy as np

if "/opt/trn_rl_repo" not in sys.path:
    sys.path.insert(0, "/opt/trn_rl_repo")

B, Tc, Tq, H = 32, 512, 512, 1024
N_CORES = 8
B_LOCAL = B // N_CORES
KT = H // 128  # contraction tiles over h (8)
OT = H // 128  # linear-output tiles over o (8)
JT = Tq // 128  # question-token tiles (4)
IT = Tc // 128  # context-token tiles (4)
HB = H // 512  # free-dim blocks for context matmul (2)
SCALE = 1.0 / 32.0  # 1/sqrt(H)
NEG = -1.0e7


def _build_program(b_local: int, use_f32r: bool = True):
    import concourse.bacc as bacc
    import concourse.mybir as mybir
    import concourse.tile as tile

    f32 = mybir.dt.float32
    # reduced-precision single-pass fp32 matmul format; every tensor feeding
    # an fp32r matmul must itself be typed fp32r end-to-end (BIR verifier)
    mdt = mybir.dt.float32r if use_f32r else mybir.dt.float32

    nc = bacc.Bacc("TRN2", debug=False)

    oqT_d = nc.declare_dram_parameter("oqT", [b_local, H, Tq], mdt, isOutput=False)
    ocT_d = nc.declare_dram_parameter("ocT", [b_local, H, Tc], mdt, isOutput=False)
    oqN_d = nc.declare_dram_parameter("oqN", [b_local, Tq, H], mdt, isOutput=False)
    wt_d = nc.declare_dram_parameter("wt", [H, H], mdt, isOutput=False)  # W.T [h, o]
    bias_d = nc.declare_dram_parameter("biasP", [128, OT], f32, isOutput=False)
    qb_d = nc.declare_dram_parameter("qb", [b_local, 128, JT], f32, isOutput=False)
    ones_d = nc.declare_dram_parameter("onesP", [128, 1], mdt, isOutput=False)
    out_d = nc.declare_dram_parameter("out", [b_local, Tc, H], f32, isOutput=True)

    with tile.TileContext(nc) as tc:
        with (
            tc.tile_pool(name="const", bufs=1) as cpool,
            tc.tile_pool(name="inp", bufs=2) as ipool,
            tc.tile_pool(name="work", bufs=1) as wpool,
            tc.tile_pool(name="outp", bufs=3) as opool,
            tc.tile_pool(name="ps_u", bufs=2, space="PSUM") as ps_u,
            tc.tile_pool(name="ps_s", bufs=2, space="PSUM") as ps_s,
            tc.tile_pool(name="ps_c", bufs=2, space="PSUM") as ps_c,
            tc.tile_pool(name="ps_d", bufs=1, space="PSUM") as ps_d,
            tc.tile_pool(name="ps_dc", bufs=1, space="PSUM") as ps_dc,
        ):
            wt = cpool.tile([128, KT, H], mdt)
            for k in range(KT):
                nc.sync.dma_start(out=wt[:, k, :], in_=wt_d[k * 128 : (k + 1) * 128, :])
            biasP = cpool.tile([128, OT], f32)
            nc.sync.dma_start(out=biasP, in_=bias_d[:, :])
            ones = cpool.tile([128, 1], mdt)
            nc.sync.dma_start(out=ones, in_=ones_d[:, :])
            ones_s = cpool.tile([1, 1], f32)
            nc.vector.memset(ones_s, 1.0)

            for b in range(b_local):
                oqT = ipool.tile([128, KT, Tq], mdt, tag="oqT")
                ocT = ipool.tile([128, KT, Tc], mdt, tag="ocT")
                oqN = ipool.tile([128, JT, H], mdt, tag="oqN")
                qb = ipool.tile([128, JT], f32, tag="qb")
                for k in range(KT):
                    nc.sync.dma_start(
                        out=oqT[:, k, :], in_=oqT_d[b, k * 128 : (k + 1) * 128, :]
                    )
                for k in range(KT):
                    nc.sync.dma_start(
                        out=ocT[:, k, :], in_=ocT_d[b, k * 128 : (k + 1) * 128, :]
                    )
                for j in range(JT):
                    nc.sync.dma_start(
                        out=oqN[:, j, :], in_=oqN_d[b, j * 128 : (j + 1) * 128, :]
                    )
                nc.sync.dma_start(out=qb, in_=qb_d[b])

                # ---- Linear: u[o, j] = (W @ o_q.T + b) / 32 ----
                u = wpool.tile([128, OT, Tq], mdt, tag="u")
                for o in range(OT):
                    ups = ps_u.tile([128, Tq], f32, tag="ups")
                    for k in range(KT):
                        nc.tensor.matmul(
                            ups,
                            wt[:, k, o * 128 : (o + 1) * 128],
                            oqT[:, k, :],
                            start=(k == 0),
                            stop=(k == KT - 1),
                        )
                    nc.any.tensor_scalar(
                        out=u[:, o, :],
                        in0=ups,
                        scalar1=biasP[:, o : o + 1],
                        scalar2=SCALE,
                        op0=mybir.AluOpType.add,
                        op1=mybir.AluOpType.mult,
                    )

                # ---- score_T + exp: e[j, i] = exp(u.T @ o_cT + qbias[j]) ----
                e_tiles = []
                for jt in range(JT):
                    sps = ps_s.tile([128, Tc], f32, tag="sps")
                    for o in range(OT):
                        nc.tensor.matmul(
                            sps,
                            u[:, o, jt * 128 : (jt + 1) * 128],
                            ocT[:, o, :],
                            start=(o == 0),
                            stop=(o == OT - 1),
                        )
                    e = wpool.tile([128, Tc], mdt, tag=f"e{jt}")
                    nc.scalar.activation(
                        out=e,
                        in_=sps,
                        func=mybir.ActivationFunctionType.Exp,
                        bias=qb[:, jt : jt + 1],
                        scale=1.0,
                    )
                    e_tiles.append(e)

                # ---- denominator d[1, i] = sum_j e[j, i] ----
                dps = ps_d.tile([1, Tc], f32, tag="dps")
                for jt in range(JT):
                    nc.tensor.matmul(
                        dps,
                        ones,
                        e_tiles[jt],
                        start=(jt == 0),
                        stop=(jt == JT - 1),
                    )
                dsb = wpool.tile([1, Tc], f32, tag="dsb")
                nc.vector.tensor_copy(out=dsb, in_=dps)

                # transpose 1/d to per-partition columns via K=1 matmuls
                r_cols = []
                for it in range(IT):
                    dcps = ps_dc.tile([128, 1], f32, tag="dcps")
                    nc.tensor.matmul(
                        dcps,
                        dsb[:, it * 128 : (it + 1) * 128],
                        ones_s[0:1, 0:1],
                        start=True,
                        stop=True,
                    )
                    r = wpool.tile([128, 1], f32, tag=f"r{it}")
                    nc.vector.reciprocal(out=r, in_=dcps)
                    r_cols.append(r)

                # ---- context[i, h] = (e / d).T @ o_q ----
                for it in range(IT):
                    osb = opool.tile([128, H], f32, tag="osb")
                    for hb in range(HB):
                        cps = ps_c.tile([128, 512], f32, tag="cps")
                        for jt in range(JT):
                            nc.tensor.matmul(
                                cps,
                                e_tiles[jt][:, it * 128 : (it + 1) * 128],
                                oqN[:, jt, hb * 512 : (hb + 1) * 512],
                                start=(jt == 0),
                                stop=(jt == JT - 1),
                            )
                        nc.any.tensor_scalar(
                            out=osb[:, hb * 512 : (hb + 1) * 512],
                            in0=cps,
                            scalar1=r_cols[it],
                            scalar2=None,
                            op0=mybir.AluOpType.mult,
                        )
                    nc.sync.dma_start(
                        out=out_d[b, it * 128 : (it + 1) * 128, :], in_=osb
                    )

    nc.compile()
    return nc


def _host_inputs(o_c, o_q, W, b, q_lengths):
    """Build the per-core input maps (host-side sharding + re-layout)."""
    wt_host = np.ascontiguousarray(W.T)  # [h, o]
    bias_host = np.ascontiguousarray(b.reshape(OT, 128).T)  # [128, ot]
    # exp-bias: 0 for valid j, -1e7 for padded j; laid out [p, jt], j = jt*128+p
    jidx = np.arange(JT)[None, :] * 128 + np.arange(128)[:, None]  # [128, JT]
    in_maps = []
    for c in range(N_CORES):
        sl = slice(c * B_LOCAL, (c + 1) * B_LOCAL)
        oq = np.ascontiguousarray(o_q[sl])
        oqT = np.ascontiguousarray(o_q[sl].transpose(0, 2, 1))
        ocT = np.ascontiguousarray(o_c[sl].transpose(0, 2, 1))
        qb = np.empty((B_LOCAL, 128, JT), np.float32)
        for lb in range(B_LOCAL):
            ql = int(q_lengths[c * B_LOCAL + lb])
            qb[lb] = np.where(jidx < ql, np.float32(0.0), np.float32(NEG))
        in_maps.append(
            {
                "oqT": oqT,
                "ocT": ocT,
                "oqN": oq,
                "wt": wt_host,
                "biasP": bias_host,
                "qb": qb,
                "onesP": np.ones((128, 1), np.float32),
            }
        )
    return in_maps


def kernel(**inputs) -> np.ndarray:
    o_c = np.asarray(inputs["o_c"], dtype=np.float32)
    o_q = np.asarray(inputs["o_q"], dtype=np.float32)
    W = np.asarray(inputs["W"], dtype=np.float32)
    b = np.asarray(inputs["b"], dtype=np.float32)
    q_lengths = np.asarray(inputs["q_lengths"]).astype(np.int64)
    c_lengths = np.asarray(inputs["c_lengths"]).astype(np.int64)

    from concourse.bass_utils import run_bass_kernel_spmd

    in_maps = _host_inputs(o_c, o_q, W, b, q_lengths)
    nc = _build_program(B_LOCAL)

    trace = bool(int(os.environ.get("KERNEL_TRACE", "0")))
    res = run_bass_kernel_spmd(
        nc, in_maps, core_ids=list(range(N_CORES)), trace=trace
    )
    if trace:
        kernel.last_results = res

    out = np.zeros((B, Tc, H), dtype=np.float32)
    for c in range(N_CORES):
        dev = res.results[c]["out"]
        for lb in range(B_LOCAL):
            g = c * B_LOCAL + lb
            cl = int(c_lengths[g])
            out[g, :cl] = dev[lb, :cl]
    return out
